# revision 2
# baseline (speedup 1.0000x reference)
"""Trainium2 Bass kernel for nn_Damping_layer: out = kipf_term - lbda[:, None] * input_term.

Pure row-parallel over 8 NeuronCores (12500 rows/core, host-padded to
12544 = 7 tiles x [128 partitions x 14 rows]). input and kipf are
host-interleaved into one DRAM tensor z laid out [t, p, h, j, c], so each
partition holds one 28 KiB contiguous run per tile — a single maximal
descriptor per (partition, tile), which measures 26.9 GB/s/engine vs
26.35 at 7 KiB. Each tile is one 3.5 MiB load; each output tile one
1.75 MiB store.

All 7 loads are emitted before any store (alternating the two HWDGE
rings) so no compute-gated store can head-of-line-block the load front;
stores ride the rings behind the loads on opposite parity. lbda is
host-negated and pre-shuffled to the [partition, group] layout and rides
SWDGE (gpsimd), keeping both HWDGE head slots for the first big loads.
Compute is one fused DVE op per 256-col row group:
    out = (input * (-lbda)) + kipf            (scalar_tensor_tensor)
"""

import numpy as np

N_NODES = 100000
N_FEAT = 256
N_CORES = 8
ROWS_PER_CORE = N_NODES // N_CORES  # 12500

R_PP = 14                       # rows per partition in a tile
TILE_ROWS = 128 * R_PP          # 1792 rows per tile
N_TILES = 7                     # tiles per core
PAD_ROWS = N_TILES * TILE_ROWS  # 12544 rows per core after padding
LB_COLS = N_TILES * R_PP        # 98

_CACHE = {}


def _build_nc():
    from contextlib import ExitStack

    import concourse.bacc as bacc
    import concourse.mybir as mybir
    import concourse.tile as tile

    FP32 = mybir.dt.float32
    MULT = mybir.AluOpType.mult
    ADD = mybir.AluOpType.add
    nc = bacc.Bacc(
        "TRN2", target_bir_lowering=False, debug=False, num_devices=N_CORES
    )
    z = nc.dram_tensor(
        "z", [2 * PAD_ROWS, N_FEAT], FP32, kind="ExternalInput"
    ).ap()
    lb = nc.dram_tensor("lb", [128, LB_COLS], FP32, kind="ExternalInput").ap()
    o = nc.dram_tensor("o", [PAD_ROWS, N_FEAT], FP32, kind="ExternalOutput").ap()

    # z layout (host-built): [t, p, h, j, c] with h=0 input rows, h=1 kipf
    # rows; partition p holds one 2*R_PP KiB (28 KiB) contiguous DRAM run
    # per tile — a single descriptor per (partition, tile).
    zv = z.rearrange(
        "(t p h j) c -> t p (h j c)", t=N_TILES, h=2, p=128, j=R_PP
    )
    ov = o.rearrange("(t p j) c -> t p (j c)", t=N_TILES, p=128, j=R_PP)

    KOFF = R_PP * N_FEAT  # kipf half offset within a z tile

    with tile.TileContext(nc) as tc, ExitStack() as ctx:
        const = ctx.enter_context(tc.tile_pool(name="const", bufs=1))
        zpool = ctx.enter_context(tc.tile_pool(name="zp", bufs=4))
        opool = ctx.enter_context(tc.tile_pool(name="op", bufs=6))

        # lbda rides SWDGE (gpsimd) so both HWDGE rings' head slots go to
        # the first big z loads.
        lbt = const.tile([128, LB_COLS], FP32)
        nc.gpsimd.dma_start(out=lbt[:], in_=lb[:])

        zts = []
        for t in range(N_TILES):
            zt = zpool.tile([128, 2 * R_PP * N_FEAT], FP32, tag="zt")
            eng = nc.sync if t % 2 == 0 else nc.scalar
            eng.dma_start(out=zt[:], in_=zv[t])
            zts.append(zt)

        for t in range(N_TILES):
            zt = zts[t]
            ot = opool.tile([128, R_PP * N_FEAT], FP32, tag="ot")
            for j in range(R_PP):
                s = slice(j * N_FEAT, (j + 1) * N_FEAT)
                sk = slice(KOFF + j * N_FEAT, KOFF + (j + 1) * N_FEAT)
                c = t * R_PP + j
                nc.vector.scalar_tensor_tensor(
                    out=ot[:, s],
                    in0=zt[:, s],
                    scalar=lbt[:, c : c + 1],
                    in1=zt[:, sk],
                    op0=MULT,
                    op1=ADD,
                )
            eng = nc.scalar if t % 2 == 0 else nc.sync
            eng.dma_start(out=ov[t], in_=ot[:])

    nc.compile()
    return nc


def _get_nc():
    if "nc" not in _CACHE:
        _CACHE["nc"] = _build_nc()
    return _CACHE["nc"]


def _shuffle_neg_lbda(lb_core):
    """[PAD_ROWS] -> [128, LB_COLS] with lb[p, t*R_PP+j] = -lbda[t*1792 + p*R_PP + j]."""
    return np.ascontiguousarray(
        -lb_core.reshape(N_TILES, 128, R_PP)
        .transpose(1, 0, 2)
        .reshape(128, LB_COLS)
    )


def _make_in_maps(input_term, kipf_term, lbda):
    input_term = np.asarray(input_term, dtype=np.float32)
    kipf_term = np.asarray(kipf_term, dtype=np.float32)
    lbda = np.asarray(lbda, dtype=np.float32)
    in_maps = []
    for c in range(N_CORES):
        sl = slice(c * ROWS_PER_CORE, (c + 1) * ROWS_PER_CORE)
        xpadded = np.zeros((PAD_ROWS, N_FEAT), np.float32)
        xpadded[:ROWS_PER_CORE] = input_term[sl]
        kpadded = np.zeros((PAD_ROWS, N_FEAT), np.float32)
        kpadded[:ROWS_PER_CORE] = kipf_term[sl]
        zc = np.empty((N_TILES, 128, 2, R_PP, N_FEAT), np.float32)
        zc[:, :, 0] = xpadded.reshape(N_TILES, 128, R_PP, N_FEAT)
        zc[:, :, 1] = kpadded.reshape(N_TILES, 128, R_PP, N_FEAT)
        lpadded = np.zeros((PAD_ROWS,), np.float32)
        lpadded[:ROWS_PER_CORE] = lbda[sl]
        in_maps.append(
            {
                "z": zc.reshape(2 * PAD_ROWS, N_FEAT),
                "lb": _shuffle_neg_lbda(lpadded),
            }
        )
    return in_maps


def kernel(input_term, kipf_term, lbda, spar=None, **_unused):
    from concourse.bass_utils import run_bass_kernel_spmd

    nc = _get_nc()
    in_maps = _make_in_maps(input_term, kipf_term, lbda)
    res = run_bass_kernel_spmd(nc, in_maps, list(range(N_CORES))).results
    return np.concatenate(
        [res[c]["o"][:ROWS_PER_CORE] for c in range(N_CORES)], axis=0
    )


# revision 3
# speedup vs baseline: 1.0053x; 1.0053x over previous
"""Trainium2 Bass kernel for nn_Damping_layer: out = kipf_term - lbda[:, None] * input_term.

Row-parallel over 8 cores (12500 rows/core, host-padded to 12544 =
7 tiles x [128 partitions x 14 rows]). input and kipf host-interleaved
into one DRAM tensor z laid out [t, p, h, j, c]: one 28 KiB contiguous
run per (partition, tile) — a single maximal descriptor each.

Compute is IN-PLACE: each fused DVE op overwrites the input slot with
(input * -lbda) + kipf, so there is no separate output pool and all 7
z tiles (196 KiB/partition) are SBUF-resident at once. Every load
dispatches immediately (no pool-recycle gating anywhere); the only
dependencies are load-complete -> compute -> store. Stores read the
input half back out (14 KiB runs). Loads/stores alternate the two HWDGE
rings on opposite parity; lbda is host-negated, pre-shuffled, and rides
SWDGE so both HWDGE head slots go to the first big loads.
"""

import numpy as np

N_NODES = 100000
N_FEAT = 256
N_CORES = 8
ROWS_PER_CORE = N_NODES // N_CORES  # 12500

R_PP = 14                       # rows per partition in a tile
TILE_ROWS = 128 * R_PP          # 1792 rows per tile
N_TILES = 7                     # tiles per core
PAD_ROWS = N_TILES * TILE_ROWS  # 12544 rows per core after padding
LB_COLS = N_TILES * R_PP        # 98

_CACHE = {}


def _build_nc():
    from contextlib import ExitStack

    import concourse.bacc as bacc
    import concourse.mybir as mybir
    import concourse.tile as tile

    FP32 = mybir.dt.float32
    MULT = mybir.AluOpType.mult
    ADD = mybir.AluOpType.add
    nc = bacc.Bacc(
        "TRN2", target_bir_lowering=False, debug=False, num_devices=N_CORES
    )
    z = nc.dram_tensor(
        "z", [2 * PAD_ROWS, N_FEAT], FP32, kind="ExternalInput"
    ).ap()
    lb = nc.dram_tensor("lb", [128, LB_COLS], FP32, kind="ExternalInput").ap()
    o = nc.dram_tensor("o", [PAD_ROWS, N_FEAT], FP32, kind="ExternalOutput").ap()

    zv = z.rearrange(
        "(t p h j) c -> t p (h j c)", t=N_TILES, h=2, p=128, j=R_PP
    )
    ov = o.rearrange("(t p j) c -> t p (j c)", t=N_TILES, p=128, j=R_PP)

    KOFF = R_PP * N_FEAT  # kipf half offset within a z tile

    with tile.TileContext(nc) as tc, ExitStack() as ctx:
        const = ctx.enter_context(tc.tile_pool(name="const", bufs=1))
        zpool = ctx.enter_context(tc.tile_pool(name="zp", bufs=N_TILES))

        lbt = const.tile([128, LB_COLS], FP32)
        nc.gpsimd.dma_start(out=lbt[:], in_=lb[:])

        zts = []
        for t in range(N_TILES):
            zt = zpool.tile([128, 2 * R_PP * N_FEAT], FP32, tag="zt")
            eng = nc.sync if t % 2 == 0 else nc.scalar
            eng.dma_start(out=zt[:], in_=zv[t])
            zts.append(zt)

        for t in range(N_TILES):
            zt = zts[t]
            for j in range(R_PP):
                s = slice(j * N_FEAT, (j + 1) * N_FEAT)
                sk = slice(KOFF + j * N_FEAT, KOFF + (j + 1) * N_FEAT)
                c = t * R_PP + j
                nc.vector.scalar_tensor_tensor(
                    out=zt[:, s],
                    in0=zt[:, s],
                    scalar=lbt[:, c : c + 1],
                    in1=zt[:, sk],
                    op0=MULT,
                    op1=ADD,
                )
            eng = nc.scalar if t % 2 == 0 else nc.sync
            eng.dma_start(out=ov[t], in_=zt[:, :KOFF])

    nc.compile()
    return nc


def _get_nc():
    if "nc" not in _CACHE:
        _CACHE["nc"] = _build_nc()
    return _CACHE["nc"]


def _shuffle_neg_lbda(lb_core):
    """[PAD_ROWS] -> [128, LB_COLS] with lb[p, t*R_PP+j] = -lbda[t*1792 + p*R_PP + j]."""
    return np.ascontiguousarray(
        -lb_core.reshape(N_TILES, 128, R_PP)
        .transpose(1, 0, 2)
        .reshape(128, LB_COLS)
    )


def _make_in_maps(input_term, kipf_term, lbda):
    input_term = np.asarray(input_term, dtype=np.float32)
    kipf_term = np.asarray(kipf_term, dtype=np.float32)
    lbda = np.asarray(lbda, dtype=np.float32)
    in_maps = []
    for c in range(N_CORES):
        sl = slice(c * ROWS_PER_CORE, (c + 1) * ROWS_PER_CORE)
        xpadded = np.zeros((PAD_ROWS, N_FEAT), np.float32)
        xpadded[:ROWS_PER_CORE] = input_term[sl]
        kpadded = np.zeros((PAD_ROWS, N_FEAT), np.float32)
        kpadded[:ROWS_PER_CORE] = kipf_term[sl]
        zc = np.empty((N_TILES, 128, 2, R_PP, N_FEAT), np.float32)
        zc[:, :, 0] = xpadded.reshape(N_TILES, 128, R_PP, N_FEAT)
        zc[:, :, 1] = kpadded.reshape(N_TILES, 128, R_PP, N_FEAT)
        lpadded = np.zeros((PAD_ROWS,), np.float32)
        lpadded[:ROWS_PER_CORE] = lbda[sl]
        in_maps.append(
            {
                "z": zc.reshape(2 * PAD_ROWS, N_FEAT),
                "lb": _shuffle_neg_lbda(lpadded),
            }
        )
    return in_maps


def kernel(input_term, kipf_term, lbda, spar=None, **_unused):
    from concourse.bass_utils import run_bass_kernel_spmd

    nc = _get_nc()
    in_maps = _make_in_maps(input_term, kipf_term, lbda)
    res = run_bass_kernel_spmd(nc, in_maps, list(range(N_CORES))).results
    return np.concatenate(
        [res[c]["o"][:ROWS_PER_CORE] for c in range(N_CORES)], axis=0
    )
